# revision 7
# baseline (speedup 1.0000x reference)
"""TRN2 Bass kernel for nn_CycleEmbedding0 (segment_reduce).

out[c, :] = sum_{e: a1[e]==c} emb[x[a0[e]], :]   c in [0, 500000), emb [28,128]

Sharding (per hint "shard segments"): cycles split 62500/core across 8
NeuronCores; host groups each core's edges by 512-cycle block (layout /
sharding only), device does the numeric work per 128-edge chunk:
  onehot_kT [28,128e] = (iota_p == k_row)          DVE
  gathered  [128e,128h] = onehot_kT.T @ emb        PE matmul (PSUM)
  onehot_c  [128e,512c] = (c_rel == iota_row)      DVE
  outT_blk  [128h,512c] += gathered.T-contracted   PE matmul, PSUM
                           (lhsT=gathered_bf16, rhs=onehot_c)
Per block: copy PSUM -> SBUF -> DMA to HBM [123,128,512]; host folds the
transposed blocks back into [500000,128].

kernel.py is self-contained (shapes hardcoded, no sibling imports).
"""
import sys
import numpy as np

sys.path.insert(0, "/opt/trn_rl_repo")

NUM_ATOM_TYPES = 28
HID = 128
NUM_CYCLES = 500_000
N_CORES = 8
CYC_PER_CORE = NUM_CYCLES // N_CORES      # 62500
BLK = 512
NBLK = (CYC_PER_CORE + BLK - 1) // BLK    # 123
CHUNK = 128

_compiled = {}


def _host_prep(x, atom_to_cycle):
    """Shard + block-group + pad (layout). Also composes k=x[a0] lookup
    while reordering the edge list (index plumbing on host; the actual
    embedding lookup/summation math runs on device)."""
    a0 = atom_to_cycle[0].astype(np.int64)
    a1 = atom_to_cycle[1].astype(np.int32)
    k_all = np.asarray(x).astype(np.int32)[a0]     # per-edge atom type
    order = np.argsort(a1, kind="stable")
    ks, a1s = k_all[order], a1[order]

    cores = []
    max_chunks = 0
    bounds = np.searchsorted(a1s, np.arange(N_CORES + 1) * CYC_PER_CORE)
    for c in range(N_CORES):
        lo, hi = bounds[c], bounds[c + 1]
        ck, ca1 = ks[lo:hi], a1s[lo:hi] - c * CYC_PER_CORE
        blk = ca1 // BLK
        cnt = np.bincount(blk, minlength=NBLK)
        pad_cnt = np.maximum((cnt + CHUNK - 1) // CHUNK, 1) * CHUNK
        tot = int(pad_cnt.sum())
        pk = np.full(tot, -1, np.int32)
        prel = np.full(tot, -1, np.int32)
        starts = np.zeros(NBLK + 1, np.int64)
        np.cumsum(pad_cnt, out=starts[1:])
        bstart = np.zeros(NBLK + 1, np.int64)
        np.cumsum(cnt, out=bstart[1:])
        for b in range(NBLK):
            n, s = int(cnt[b]), int(starts[b])
            pk[s:s + n] = ck[bstart[b]:bstart[b] + n]
            prel[s:s + n] = ca1[bstart[b]:bstart[b] + n] - b * BLK
        cores.append((pk, prel, pad_cnt // CHUNK))
        max_chunks = max(max_chunks, tot // CHUNK)
    return cores, max_chunks


def _build(n_chunks, chunks_per_block):
    """Build + compile the SPMD bass program. chunks_per_block: [NBLK]
    arrays identical across cores after padding to max_chunks (we pad
    the per-block chunk counts so every core shares one program)."""
    import concourse.mybir as mybir
    import concourse.tile as tile
    from concourse import bacc

    F32 = mybir.dt.float32
    BF16 = mybir.dt.float16
    I32 = mybir.dt.int32

    nc = bacc.Bacc("TRN2", target_bir_lowering=False, debug=False,
                   num_devices=N_CORES)
    khot_t = nc.dram_tensor("khot", [NUM_ATOM_TYPES, n_chunks * CHUNK], BF16,
                            kind="ExternalInput")
    crel_t = nc.dram_tensor("crel", [128, n_chunks], F32, kind="ExternalInput")
    emb_t = nc.dram_tensor("emb", [NUM_ATOM_TYPES, HID], F32, kind="ExternalInput")
    out_t = nc.dram_tensor("out", [NBLK, HID * BLK], F32, kind="ExternalOutput")

    SUP = 64  # chunks per k staging superblock
    with tile.TileContext(nc) as tc:
        with (
            tc.tile_pool(name="const", bufs=1) as cpool,
            tc.tile_pool(name="stage", bufs=2) as spool,
            tc.tile_pool(name="work", bufs=3) as wpool,
            tc.tile_pool(name="oh", bufs=3) as ohpool,
            tc.tile_pool(name="psA", bufs=2, space="PSUM") as psa,
            tc.tile_pool(name="psB", bufs=2, space="PSUM") as psb,
            tc.tile_pool(name="osb", bufs=2) as opool,
        ):
            emb_sb = cpool.tile([NUM_ATOM_TYPES, HID], BF16)
            emb_f32 = cpool.tile([NUM_ATOM_TYPES, HID], F32)
            nc.sync.dma_start(out=emb_f32[:], in_=emb_t[:])
            nc.vector.tensor_copy(out=emb_sb[:], in_=emb_f32[:])
            iota512 = cpool.tile([128, BLK], BF16)
            nc.gpsimd.iota(iota512[:], pattern=[[1, BLK]], base=0,
                           channel_multiplier=0,
                           allow_small_or_imprecise_dtypes=True)

            # block -> list of global chunk ids
            chunk_of_block = []
            cid = 0
            for b in range(NBLK):
                chunk_of_block.append(list(range(cid, cid + chunks_per_block[b])))
                cid += chunks_per_block[b]
            assert cid == n_chunks

            # crel resident, converted to fp16 (exact for values <= 2048)
            crel_f32 = cpool.tile([128, n_chunks], F32)
            nc.sync.dma_start(out=crel_f32[:], in_=crel_t[:])
            crel_sb = cpool.tile([128, n_chunks], BF16)
            nc.vector.tensor_copy(out=crel_sb[:], in_=crel_f32[:])

            cur_k = None  # staged k superblock, double-buffered via pool

            for b in range(NBLK):
                pb = psb.tile([HID, BLK], F32, space="PSUM")
                for li, ci in enumerate(chunk_of_block[b]):
                    if ci % SUP == 0:
                        w = min(SUP, n_chunks - ci)
                        cur_k = spool.tile(
                            [NUM_ATOM_TYPES, SUP * CHUNK], BF16, tag="kstage")
                        nc.sync.dma_start(
                            out=cur_k[:, :w * CHUNK],
                            in_=khot_t[:, ci * CHUNK:(ci + w) * CHUNK])
                    off = ci % SUP
                    ohk = cur_k[:, off * CHUNK:(off + 1) * CHUNK]
                    cr = crel_sb[:, ci:ci + 1]
                    pa = psa.tile([CHUNK, HID], F32, space="PSUM")
                    nc.tensor.matmul(pa[:], lhsT=ohk, rhs=emb_sb[:],
                                     start=True, stop=True)
                    gat = wpool.tile([CHUNK, HID], BF16, tag="gat")
                    nc.vector.tensor_copy(out=gat[:], in_=pa[:])
                    ohc = ohpool.tile([CHUNK, BLK], BF16, tag="ohc")
                    nc.vector.tensor_tensor(
                        out=ohc[:], in0=cr.to_broadcast([CHUNK, BLK]),
                        in1=iota512[:],
                        op=mybir.AluOpType.is_equal)
                    nc.tensor.matmul(pb[:], lhsT=gat[:], rhs=ohc[:],
                                     start=(li == 0),
                                     stop=(li == len(chunk_of_block[b]) - 1))
                ob = opool.tile([HID, BLK], F32, tag="ob")
                nc.vector.tensor_copy(out=ob[:], in_=pb[:])
                nc.sync.dma_start(
                    out=out_t[b:b + 1, :].rearrange("o (h c) -> (o h) c", h=HID),
                    in_=ob[:])
    nc.compile()
    return nc


def kernel(x, atom_to_cycle, emb_weight):
    from concourse.bass_utils import run_bass_kernel_spmd

    cores, n_chunks = _host_prep(x, atom_to_cycle)
    emb = np.asarray(emb_weight, np.float32)

    # unify per-core chunk counts per block so one program serves all
    cpb = np.zeros(NBLK, np.int64)
    for pk, prel, cb in cores:
        cpb = np.maximum(cpb, cb)
    n_chunks = int(cpb.sum())

    key = (n_chunks, tuple(cpb))
    if key not in _compiled:
        _compiled[key] = _build(n_chunks, cpb.astype(int))
    nc = _compiled[key]

    import ml_dtypes
    in_maps = []
    for pk, prel, cb in cores:
        E = n_chunks * CHUNK
        kf = np.full(E, -1, np.int32)
        cf = np.full((128, n_chunks), -1.0, np.float32)
        src_start = np.zeros(NBLK + 1, np.int64)
        np.cumsum(cb * CHUNK, out=src_start[1:])
        dst_start = np.zeros(NBLK + 1, np.int64)
        np.cumsum(cpb * CHUNK, out=dst_start[1:])
        for b in range(NBLK):
            n = int(cb[b]) * CHUNK
            s, d = int(src_start[b]), int(dst_start[b])
            kf[d:d + n] = pk[s:s + n]
            col = prel[s:s + n].astype(np.float32).reshape(-1, CHUNK).T
            cf[:, d // CHUNK:d // CHUNK + n // CHUNK] = col
        khot = (kf.reshape(1, -1) ==
                np.arange(NUM_ATOM_TYPES).reshape(NUM_ATOM_TYPES, 1))
        khot = khot.astype(np.float16)
        in_maps.append({"khot": khot, "crel": cf, "emb": emb})

    res = run_bass_kernel_spmd(nc, in_maps, list(range(N_CORES)))
    out = np.empty((NUM_CYCLES, HID), np.float32)
    for c in range(N_CORES):
        blk = res.results[c]["out"].reshape(NBLK, HID, BLK)
        full = blk.transpose(0, 2, 1).reshape(NBLK * BLK, HID)
        out[c * CYC_PER_CORE:(c + 1) * CYC_PER_CORE] = full[:CYC_PER_CORE]
    return out


# revision 8
# speedup vs baseline: 87.3065x; 87.3065x over previous
"""TRN2 Bass kernel for nn_CycleEmbedding0 (segment_reduce).

out[c, :] = sum_{e: a1[e]==c} emb[x[a0[e]], :]   c in [0, 500000), emb [28,128]

Sharding (per hint "shard segments"): cycles split 62500/core across 8
NeuronCores; host groups each core's edges by 512-cycle block (layout /
sharding only), device does the numeric work per 128-edge chunk:
  onehot_kT [28,128e] = (iota_p == k_row)          DVE
  gathered  [128e,128h] = onehot_kT.T @ emb        PE matmul (PSUM)
  onehot_c  [128e,512c] = (c_rel == iota_row)      DVE
  outT_blk  [128h,512c] += gathered.T-contracted   PE matmul, PSUM
                           (lhsT=gathered_bf16, rhs=onehot_c)
Per block: copy PSUM -> SBUF -> DMA to HBM [123,128,512]; host folds the
transposed blocks back into [500000,128].

kernel.py is self-contained (shapes hardcoded, no sibling imports).
"""
import sys
import numpy as np

sys.path.insert(0, "/opt/trn_rl_repo")

NUM_ATOM_TYPES = 28
HID = 128
NUM_CYCLES = 500_000
N_CORES = 8
CYC_PER_CORE = NUM_CYCLES // N_CORES      # 62500
BLK = 512
NBLK = (CYC_PER_CORE + BLK - 1) // BLK    # 123
CHUNK = 128

_compiled = {}


def _host_prep(x, atom_to_cycle):
    """Shard + block-group + pad (layout). Also composes k=x[a0] lookup
    while reordering the edge list (index plumbing on host; the actual
    embedding lookup/summation math runs on device)."""
    a0 = atom_to_cycle[0].astype(np.int64)
    a1 = atom_to_cycle[1].astype(np.int32)
    k_all = np.asarray(x).astype(np.int32)[a0]     # per-edge atom type
    order = np.argsort(a1, kind="stable")
    ks, a1s = k_all[order], a1[order]

    cores = []
    max_chunks = 0
    bounds = np.searchsorted(a1s, np.arange(N_CORES + 1) * CYC_PER_CORE)
    for c in range(N_CORES):
        lo, hi = bounds[c], bounds[c + 1]
        ck, ca1 = ks[lo:hi], a1s[lo:hi] - c * CYC_PER_CORE
        blk = ca1 // BLK
        cnt = np.bincount(blk, minlength=NBLK)
        pad_cnt = np.maximum((cnt + CHUNK - 1) // CHUNK, 1) * CHUNK
        tot = int(pad_cnt.sum())
        pk = np.full(tot, -1, np.int32)
        prel = np.full(tot, -1, np.int32)
        starts = np.zeros(NBLK + 1, np.int64)
        np.cumsum(pad_cnt, out=starts[1:])
        bstart = np.zeros(NBLK + 1, np.int64)
        np.cumsum(cnt, out=bstart[1:])
        for b in range(NBLK):
            n, s = int(cnt[b]), int(starts[b])
            pk[s:s + n] = ck[bstart[b]:bstart[b] + n]
            prel[s:s + n] = ca1[bstart[b]:bstart[b] + n] - b * BLK
        cores.append((pk, prel, pad_cnt // CHUNK))
        max_chunks = max(max_chunks, tot // CHUNK)
    return cores, max_chunks


def _build(n_chunks, chunks_per_block, reps=1):
    """Build + compile the SPMD bass program. chunks_per_block: [NBLK]
    arrays identical across cores after padding to max_chunks (we pad
    the per-block chunk counts so every core shares one program)."""
    import concourse.mybir as mybir
    import concourse.tile as tile
    from concourse import bacc

    F32 = mybir.dt.float32
    BF16 = mybir.dt.float16
    I32 = mybir.dt.int32

    nc = bacc.Bacc("TRN2", target_bir_lowering=False, debug=False,
                   num_devices=N_CORES)
    khot_t = nc.dram_tensor("khot", [NUM_ATOM_TYPES, n_chunks * CHUNK], BF16,
                            kind="ExternalInput")
    crel_t = nc.dram_tensor("crel", [128, n_chunks], F32, kind="ExternalInput")
    emb_t = nc.dram_tensor("emb", [NUM_ATOM_TYPES, HID], F32, kind="ExternalInput")
    out_t = nc.dram_tensor("out", [NBLK, HID * BLK], F32, kind="ExternalOutput")

    SUP = 64  # chunks per k staging superblock
    with tile.TileContext(nc) as tc:
        with (
            tc.tile_pool(name="const", bufs=1) as cpool,
            tc.tile_pool(name="stage", bufs=2) as spool,
            tc.tile_pool(name="work", bufs=3) as wpool,
            tc.tile_pool(name="oh", bufs=3) as ohpool,
            tc.tile_pool(name="psA", bufs=2, space="PSUM") as psa,
            tc.tile_pool(name="psB", bufs=2, space="PSUM") as psb,
            tc.tile_pool(name="osb", bufs=2) as opool,
        ):
            emb_sb = cpool.tile([NUM_ATOM_TYPES, HID], BF16)
            emb_f32 = cpool.tile([NUM_ATOM_TYPES, HID], F32)
            nc.sync.dma_start(out=emb_f32[:], in_=emb_t[:])
            nc.vector.tensor_copy(out=emb_sb[:], in_=emb_f32[:])
            iota512 = cpool.tile([128, BLK], BF16)
            nc.gpsimd.iota(iota512[:], pattern=[[1, BLK]], base=0,
                           channel_multiplier=0,
                           allow_small_or_imprecise_dtypes=True)

            # block -> list of global chunk ids
            chunk_of_block = []
            cid = 0
            for b in range(NBLK):
                chunk_of_block.append(list(range(cid, cid + chunks_per_block[b])))
                cid += chunks_per_block[b]
            assert cid == n_chunks

            # crel resident, converted to fp16 (exact for values <= 2048)
            crel_f32 = cpool.tile([128, n_chunks], F32)
            nc.sync.dma_start(out=crel_f32[:], in_=crel_t[:])
            crel_sb = cpool.tile([128, n_chunks], BF16)
            nc.vector.tensor_copy(out=crel_sb[:], in_=crel_f32[:])

            cur_k = None  # staged k superblock, double-buffered via pool

            for _rep in range(reps):
              for b in range(NBLK):
                pb = psb.tile([HID, BLK], F32, space="PSUM")
                for li, ci in enumerate(chunk_of_block[b]):
                    if ci % SUP == 0:
                        w = min(SUP, n_chunks - ci)
                        cur_k = spool.tile(
                            [NUM_ATOM_TYPES, SUP * CHUNK], BF16, tag="kstage")
                        nc.sync.dma_start(
                            out=cur_k[:, :w * CHUNK],
                            in_=khot_t[:, ci * CHUNK:(ci + w) * CHUNK])
                    off = ci % SUP
                    ohk = cur_k[:, off * CHUNK:(off + 1) * CHUNK]
                    cr = crel_sb[:, ci:ci + 1]
                    pa = psa.tile([CHUNK, HID], F32, space="PSUM")
                    nc.tensor.matmul(pa[:], lhsT=ohk, rhs=emb_sb[:],
                                     start=True, stop=True)
                    gat = wpool.tile([CHUNK, HID], BF16, tag="gat")
                    nc.vector.tensor_copy(out=gat[:], in_=pa[:])
                    ohc = ohpool.tile([CHUNK, BLK], BF16, tag="ohc")
                    nc.vector.tensor_tensor(
                        out=ohc[:], in0=cr.to_broadcast([CHUNK, BLK]),
                        in1=iota512[:],
                        op=mybir.AluOpType.is_equal)
                    nc.tensor.matmul(pb[:], lhsT=gat[:], rhs=ohc[:],
                                     start=(li == 0),
                                     stop=(li == len(chunk_of_block[b]) - 1))
                ob = opool.tile([HID, BLK], F32, tag="ob")
                nc.vector.tensor_copy(out=ob[:], in_=pb[:])
                nc.sync.dma_start(
                    out=out_t[b:b + 1, :].rearrange("o (h c) -> (o h) c", h=HID),
                    in_=ob[:])
    nc.compile()
    return nc


def kernel(x, atom_to_cycle, emb_weight):
    from concourse.bass_utils import run_bass_kernel_spmd

    cores, n_chunks = _host_prep(x, atom_to_cycle)
    emb = np.asarray(emb_weight, np.float32)

    # unify per-core chunk counts per block so one program serves all
    cpb = np.zeros(NBLK, np.int64)
    for pk, prel, cb in cores:
        cpb = np.maximum(cpb, cb)
    n_chunks = int(cpb.sum())

    key = (n_chunks, tuple(cpb))
    if key not in _compiled:
        _compiled[key] = _build(n_chunks, cpb.astype(int))
    nc = _compiled[key]

    import ml_dtypes
    in_maps = []
    for pk, prel, cb in cores:
        E = n_chunks * CHUNK
        kf = np.full(E, -1, np.int32)
        cf = np.full((128, n_chunks), -1.0, np.float32)
        src_start = np.zeros(NBLK + 1, np.int64)
        np.cumsum(cb * CHUNK, out=src_start[1:])
        dst_start = np.zeros(NBLK + 1, np.int64)
        np.cumsum(cpb * CHUNK, out=dst_start[1:])
        for b in range(NBLK):
            n = int(cb[b]) * CHUNK
            s, d = int(src_start[b]), int(dst_start[b])
            kf[d:d + n] = pk[s:s + n]
            col = prel[s:s + n].astype(np.float32).reshape(-1, CHUNK).T
            cf[:, d // CHUNK:d // CHUNK + n // CHUNK] = col
        khot = (kf.reshape(1, -1) ==
                np.arange(NUM_ATOM_TYPES).reshape(NUM_ATOM_TYPES, 1))
        khot = khot.astype(np.float16)
        in_maps.append({"khot": khot, "crel": cf, "emb": emb})

    res = run_bass_kernel_spmd(nc, in_maps, list(range(N_CORES)))
    out = np.empty((NUM_CYCLES, HID), np.float32)
    for c in range(N_CORES):
        blk = res.results[c]["out"].reshape(NBLK, HID, BLK)
        full = blk.transpose(0, 2, 1).reshape(NBLK * BLK, HID)
        out[c * CYC_PER_CORE:(c + 1) * CYC_PER_CORE] = full[:CYC_PER_CORE]
    return out
